# revision 10
# baseline (speedup 1.0000x reference)
"""Trainium2 Bass kernel for CustomPointScatter (nn_CustomPointScatter).

Reference computation:
    pillar_feat = point_features.mean(axis=1)            # [40000, 64]
    out = zeros([4, 64, 512, 512]); out[b, :, y, x] = pillar_feat

Sharding: each of the 8 cores owns one output region (b, y_half) of shape
[64, 256, 512].  The host partitions pillars by destination region, folds
the mean's 1/n_points into the gather, casts to bf16 (rel err ~7e-3,
within the 2e-2 gate), pads every group to a multiple of 128, and hands
each core its pillars plus per-pillar destination row offsets.

v5 structure -- the rate-matched three-chain pipeline.  Per-core chains:
loads 21MB bf16 at ~400GB/s = 52.5us; DVE halving-add tree ~53us busy;
gpsimd SWDGE scatter stream 40 x (1.1us emission + 0.31us dispatch) plus
a ~30-90% emission slowdown while DVE is active (SBUF lockout).  HW
facts baked in (probed):
  * gpsimd indirect DMA consumes ONE offset per partition (dst(p,j) =
    offs[p,0]+j writes consecutive rows) -- so scatters are one per
    128-pillar block, and the per-entry ANT op dma_scatter_add corrupts
    rows nondeterministically on this platform.  41 -> 40 blocks via the
    actual seed-0 region max (5083 -> nmax 5120).
  * DMA completion semaphores lag data arrival by ~2-3us (engine
    spin-up + queue-wide batching): ramp to first DVE op ~12us.
Design:
  * Host pre-transposes each 2-block chunk to [point, pillar, ch] order:
    the 32-point tree is 5 contiguous stride-1 bf16 DVE ops per chunk
    (2x perf mode, 0.52ns/elem), each <=2048 elems so concurrent SWDGE
    emission is not starved; a chunk's 2 blocks become scatter-ready
    after half a tile's DVE, keeping the gpsimd stream fed.
  * The final tree level writes a persistent sums_sb buffer: scatters
    read sums_sb, not the io tile, so io buffers recycle on DVE
    completion alone and the load stream never backpressures on the
    scatter tail.
  * offs table loads via the scalar engine's separate HW queue.
  * 12 rotating output banks break same-tensor WAW serialization.
    Scatter-write packets starve behind load-read packets on the 16
    shared DMA engines while the load stream runs (bank retire lags
    8-15us), so the WAW rotation distance must exceed that lag; 8 banks
    was marginal, 12 measures best (16 regresses).  Destination cells
    are globally unique so banks have disjoint row support and the host
    sums them.  ExternalOutput DRAM arrives zero-initialised (runtime
    contract): only occupied rows are written.
The host reassembles the regions and transposes to [B, C, H, W].

Measured (8 cores, max/mean): 90.9/88.8us vs the 93.4us v1 baseline;
scatter-stream floor alone (microbench, no DVE) is 75.3us: 40 x 1.43us
emission+dispatch + 12.6us ramp + 5.9us drain/epilogue.  Rel err 7e-3.
"""

import ml_dtypes
import numpy as np

import concourse.bacc as bacc
import concourse.bass as bass
import concourse.mybir as mybir
import concourse.tile as tile
from concourse.bass_utils import run_bass_kernel_spmd

B, H, W = 4, 512, 512
N_PILLARS, N_POINTS, C = 40000, 32, 64
N_CORES = 8
P = 128
HALF = H // 2            # 256 BEV rows per core
REGION_ROWS = HALF * W   # 131072 positions per core
PAD_ROWS = P             # dump rows for padded (inactive) pillars
OUT_ROWS = REGION_ROWS + PAD_ROWS
SUP = 4                  # pillar blocks (of 128) per full super-tile
CHUNK = 2                # blocks per DVE chain / host transpose group
NBANKS = 12              # independent output tensors breaking scatter WAW chains
BUFS = 8
TAPER = 1                # taper the first/last blocks down to small tiles
BF16 = 1                 # full-bf16 pipeline (host casts; rel err ~7e-3)
TBUFS = 5                # bufs for the small taper tiles


def make_schedule(T, sup=SUP, taper=TAPER):
    """[(base_block, tile_sup), ...] covering blocks 0..T-1.

    Small tiles go FIRST (fast ramp: the first scatter is ready after a
    ~0.5MB load + one short DVE chain instead of a full super-tile) and a
    short taper goes LAST (small end-of-pipe drain)."""
    # Head taper: fast ramp (first scatter after a 0.5MB load + short DVE
    # chain).  Tail stays one s=2 tile: with sums_sb decoupling the
    # scatter stream trails DVE supply by ~6 ops anyway, so small tail
    # tiles only add DVE op overhead (each extra chunk costs ~0.7us of
    # small-op time).
    head = [1, 1, 2, 2] if taper else []
    tail = [2] if taper else []
    extra_blocks = sum(head) + sum(tail)
    while taper and (T - extra_blocks) % sup != 0:
        tail.append(1)
        extra_blocks += 1
    if not taper:
        assert T % sup == 0
    sched = []
    base = 0
    for s in head:
        sched.append((base, s))
        base += s
    for _ in range((T - extra_blocks) // sup):
        sched.append((base, sup))
        base += sup
    for s in tail:
        sched.append((base, s))
        base += s
    assert base == T
    return sched


def chunks_of(s, chunk=CHUNK):
    """Split a tile of s blocks into chunk-sized groups: [(off, cw), ...]"""
    return [(c0, min(chunk, s - c0)) for c0 in range(0, s, chunk)]


def build_nc(nmax, n_points=N_POINTS, c=C, out_rows=OUT_ROWS, sup=SUP,
             bufs=BUFS, nbanks=NBANKS, taper=TAPER, bf16=BF16, chunk=CHUNK):
    T = nmax // P          # pillar blocks
    D = n_points * c       # full row: 2048 values
    sched = make_schedule(T, sup, taper)
    dt = mybir.dt.bfloat16 if bf16 else mybir.dt.float32
    nc = bacc.Bacc("TRN2", target_bir_lowering=False)
    pf = nc.dram_tensor("pf", [nmax, D], dt, kind="ExternalInput")
    offs = nc.dram_tensor("offs", [P, T], mybir.dt.int32, kind="ExternalInput")
    banks = [
        nc.dram_tensor(f"out{k}", [out_rows, c], dt, kind="ExternalOutput")
        for k in range(nbanks)
    ]
    with tile.TileContext(nc) as tc:
        with (
            tc.tile_pool(name="io", bufs=bufs) as io_pool,
            tc.tile_pool(name="misc", bufs=1) as misc,
        ):
            offs_sb = misc.tile([P, T], mybir.dt.int32)
            sums_sb = misc.tile([P, T * c], dt)
            warm_sb = misc.tile([P, 16], dt)
            # tiny warmup DMA: wakes the 16 shared DMA engines ~2us
            # before the first data tile needs them (engine spin-up +
            # completion-sem batching otherwise delays the first DVE op
            # to ~12.3us); result unused
            nc.scalar.dma_start(out=warm_sb[:], in_=pf[0:P, 0:16])
            # scalar engine HW queue: lands early, not behind the first
            # data tiles on the sync queue
            nc.scalar.dma_start(out=offs_sb[:], in_=offs[:])
            for base, s in sched:
                rows = slice(base * P, (base + s) * P)
                sb = io_pool.tile([P, s * D], dt, tag=f"sb{s}",
                                  bufs=bufs if s == sup else TBUFS)
                # pillar j = base*128 + p*s + blk -> partition p; host
                # pre-transposed each chunk so partition p's cw*D stretch
                # is [q (32 points), blk (cw pillars), c (64 ch)]
                nc.sync.dma_start(
                    out=sb[:],
                    in_=pf[rows, :].rearrange("(p x) w -> p (x w)", x=s),
                )
                for c0, cw in chunks_of(s, chunk):
                    o0 = c0 * D
                    # 5-level halving-add tree for this chunk, contiguous
                    # bf16 ops (DVE 2x perf mode, <=2048 elems each)
                    w = cw * D // 2
                    while w > cw * c:
                        nc.vector.tensor_add(
                            out=sb[:, o0:o0 + w],
                            in0=sb[:, o0:o0 + w],
                            in1=sb[:, o0 + w:o0 + 2 * w],
                        )
                        w //= 2
                    nc.vector.tensor_add(
                        out=sums_sb[:, (base + c0) * c:(base + c0 + cw) * c],
                        in0=sb[:, o0:o0 + w],
                        in1=sb[:, o0 + w:o0 + 2 * w],
                    )
                    # one indirect DMA per 128-pillar block (HW: one
                    # offset per partition), reading the decoupled sums
                    for blk in range(c0, c0 + cw):
                        g = base + blk
                        nc.gpsimd.indirect_dma_start(
                            out=banks[g % nbanks][:],
                            out_offset=bass.IndirectOffsetOnAxis(
                                ap=offs_sb[:, g:g + 1], axis=0
                            ),
                            in_=sums_sb[:, g * c:(g + 1) * c],
                            in_offset=None,
                        )
    nc.finalize()  # Bacc.compile(): splits multi-waits for TRN2 codegen
    return nc


def shard_inputs(point_features, voxel_coords, sup=SUP, taper=TAPER,
                 bf16=BF16, chunk=CHUNK):
    pf = np.ascontiguousarray(
        np.asarray(point_features, dtype=np.float32).reshape(N_PILLARS, N_POINTS * C)
    )
    np_dt = ml_dtypes.bfloat16 if bf16 else np.float32
    vc = np.asarray(voxel_coords)
    b = vc[:, 0].astype(np.int64)
    y = vc[:, 2].astype(np.int64)
    x = vc[:, 3].astype(np.int64)
    upper = (y >= HALF).astype(np.int64)
    region = b * 2 + upper
    off = (y - upper * HALF) * W + x  # row offset within the owned region
    idx_r = [np.nonzero(region == r)[0] for r in range(N_CORES)]
    nmax = max(len(ix) for ix in idx_r)
    nmax = max(P, ((nmax + P - 1) // P) * P)
    if not taper:
        a2 = sup * P
        nmax = ((nmax + a2 - 1) // a2) * a2
    T = nmax // P
    sched = make_schedule(T, sup, taper)
    inv_np = np.float32(1.0 / N_POINTS)
    in_maps = []
    for r in range(N_CORES):
        ix = idx_r[r]
        pf_r = np.zeros((nmax, N_POINTS * C), np_dt)
        # fold the mean's 1/n_points into the gather
        pf_r[: len(ix)] = (pf[ix] * inv_np).astype(np_dt)
        offs_r = np.full(nmax, REGION_ROWS, np.int32)  # pad rows -> dump row
        offs_r[: len(ix)] = off[ix].astype(np.int32)
        # tile (base, s): pillar j = base*128 + p*s + blk -> offs_arr[p, base+blk]
        offs_arr = np.empty((P, T), np.int32)
        for base, s in sched:
            seg = offs_r[base * P:(base + s) * P].reshape(P, s)
            offs_arr[:, base:base + s] = seg
            # per chunk-of-cw blocks: transpose partition p's pillars
            # from [blk, q, c] to [q, blk, c] for contiguous DVE trees
            blockrows = pf_r[base * P:(base + s) * P].reshape(P, s, N_POINTS * C)
            for c0, cw in chunks_of(s, chunk):
                grp = blockrows[:, c0:c0 + cw].reshape(P, cw, N_POINTS, C)
                blockrows[:, c0:c0 + cw] = (
                    grp.transpose(0, 2, 1, 3).reshape(P, cw, N_POINTS * C)
                )
        in_maps.append({"pf": pf_r, "offs": np.ascontiguousarray(offs_arr)})
    return in_maps, nmax


def assemble(results):
    out = np.empty((B, C, H, W), np.float32)
    for r in range(N_CORES):
        names = sorted(results[r])       # out0..out{nbanks-1}
        region = np.asarray(results[r][names[0]], dtype=np.float32)
        for name in names[1:]:
            # banks: disjoint row support
            region = region + np.asarray(results[r][name], dtype=np.float32)
        o = region[:REGION_ROWS].reshape(HALF, W, C)
        b_, half = divmod(r, 2)
        out[b_, :, half * HALF:(half + 1) * HALF, :] = o.transpose(2, 0, 1)
    return out


def run(point_features, voxel_coords, trace=False, sup=SUP, bufs=BUFS,
        nbanks=NBANKS, taper=TAPER, bf16=BF16, chunk=CHUNK, **spmd_kwargs):
    in_maps, nmax = shard_inputs(point_features, voxel_coords,
                                 sup=sup, taper=taper, bf16=bf16, chunk=chunk)
    nc = build_nc(nmax, sup=sup, bufs=bufs, nbanks=nbanks, taper=taper,
                  bf16=bf16, chunk=chunk)
    br = run_bass_kernel_spmd(
        nc, in_maps, list(range(N_CORES)), trace=trace, **spmd_kwargs
    )
    return assemble(br.results), br


def kernel(point_features, voxel_coords):
    out, _ = run(point_features, voxel_coords)
    return out


# revision 11
# speedup vs baseline: 1.1580x; 1.1580x over previous
"""Trainium2 Bass kernel for CustomPointScatter (nn_CustomPointScatter).

Reference computation:
    pillar_feat = point_features.mean(axis=1)            # [40000, 64]
    out = zeros([4, 64, 512, 512]); out[b, :, y, x] = pillar_feat

Sharding: each of the 8 cores owns one output region (b, y_half) of shape
[64, 256, 512].  The host partitions pillars by destination region, folds
the mean's 1/n_points into the gather, casts to bf16 (rel err ~7e-3,
within the 2e-2 gate), pads every group to a multiple of 128, and hands
each core its pillars plus per-pillar destination row offsets.

v5 structure -- the rate-matched three-chain pipeline.  Per-core chains:
loads 21MB bf16 at ~400GB/s = 52.5us; DVE halving-add tree ~53us busy;
gpsimd SWDGE scatter stream 40 x (1.1us emission + 0.31us dispatch) plus
a ~30-90% emission slowdown while DVE is active (SBUF lockout).  HW
facts baked in (probed):
  * gpsimd indirect DMA consumes ONE offset per partition (dst(p,j) =
    offs[p,0]+j writes consecutive rows) -- so scatters are one per
    128-pillar block, and the per-entry ANT op dma_scatter_add corrupts
    rows nondeterministically on this platform.  41 -> 40 blocks via the
    actual seed-0 region max (5083 -> nmax 5120).
  * DMA completion semaphores lag data arrival by ~2-3us (engine
    spin-up + queue-wide batching): ramp to first DVE op ~12us.
Design:
  * Host pre-transposes each 2-block chunk to [point, pillar, ch] order:
    the 32-point tree is 5 contiguous stride-1 bf16 DVE ops per chunk
    (2x perf mode, 0.52ns/elem), each <=2048 elems so concurrent SWDGE
    emission is not starved; a chunk's 2 blocks become scatter-ready
    after half a tile's DVE, keeping the gpsimd stream fed.
  * The final tree level writes a persistent sums_sb buffer: scatters
    read sums_sb, not the io tile, so io buffers recycle on DVE
    completion alone and the load stream never backpressures on the
    scatter tail.
  * offs table loads via the scalar engine's separate HW queue.
  * 12 rotating output banks break same-tensor WAW serialization.
    Scatter-write packets starve behind load-read packets on the 16
    shared DMA engines while the load stream runs (bank retire lags
    8-15us), so the WAW rotation distance must exceed that lag; 8 banks
    was marginal, 12 measures best (16 regresses).  Destination cells
    are globally unique so banks have disjoint row support and the host
    sums them.  ExternalOutput DRAM arrives zero-initialised (runtime
    contract): only occupied rows are written.
The host reassembles the regions and transposes to [B, C, H, W].

Measured (8 cores, max/mean): 90.9/88.8us vs the 93.4us v1 baseline;
scatter-stream floor alone (microbench, no DVE) is 75.3us: 40 x 1.43us
emission+dispatch + 12.6us ramp + 5.9us drain/epilogue.  Rel err 7e-3.
"""

import ml_dtypes
import numpy as np

import concourse.bacc as bacc
import concourse.bass as bass
import concourse.mybir as mybir
import concourse.tile as tile
from concourse.bass_utils import run_bass_kernel_spmd

B, H, W = 4, 512, 512
N_PILLARS, N_POINTS, C = 40000, 32, 64
N_CORES = 8
P = 128
HALF = H // 2            # 256 BEV rows per core
REGION_ROWS = HALF * W   # 131072 positions per core
PAD_ROWS = P             # dump rows for padded (inactive) pillars
OUT_ROWS = REGION_ROWS + PAD_ROWS
SUP = 4                  # pillar blocks (of 128) per full super-tile
CHUNK = 2                # blocks per DVE chain / host transpose group
NBANKS = 12              # independent output tensors breaking scatter WAW chains
BUFS = 8
TAPER = 1                # taper the first/last blocks down to small tiles
BF16 = 1                 # full-bf16 pipeline (host casts; rel err ~7e-3)
TBUFS = 5                # bufs for the small taper tiles


def make_schedule(T, sup=SUP, taper=TAPER):
    """[(base_block, tile_sup), ...] covering blocks 0..T-1.

    Small tiles go FIRST (fast ramp: the first scatter is ready after a
    ~0.5MB load + one short DVE chain instead of a full super-tile) and a
    short taper goes LAST (small end-of-pipe drain)."""
    # Head taper: fast ramp (first scatter after a 0.5MB load + short DVE
    # chain).  Tail stays one s=2 tile: with sums_sb decoupling the
    # scatter stream trails DVE supply by ~6 ops anyway, so small tail
    # tiles only add DVE op overhead (each extra chunk costs ~0.7us of
    # small-op time).
    head = [1, 1, 2, 2] if taper else []
    tail = [2] if taper else []
    extra_blocks = sum(head) + sum(tail)
    while taper and (T - extra_blocks) % sup != 0:
        tail.append(1)
        extra_blocks += 1
    if not taper:
        assert T % sup == 0
    sched = []
    base = 0
    for s in head:
        sched.append((base, s))
        base += s
    for _ in range((T - extra_blocks) // sup):
        sched.append((base, sup))
        base += sup
    for s in tail:
        sched.append((base, s))
        base += s
    assert base == T
    return sched


def chunks_of(s, chunk=CHUNK):
    """Split a tile of s blocks into chunk-sized groups: [(off, cw), ...]"""
    return [(c0, min(chunk, s - c0)) for c0 in range(0, s, chunk)]


def build_nc(nmax, n_points=N_POINTS, c=C, out_rows=OUT_ROWS, sup=SUP,
             bufs=BUFS, nbanks=NBANKS, taper=TAPER, bf16=BF16, chunk=CHUNK):
    T = nmax // P          # pillar blocks
    D = n_points * c       # full row: 2048 values
    sched = make_schedule(T, sup, taper)
    dt = mybir.dt.bfloat16 if bf16 else mybir.dt.float32
    nc = bacc.Bacc("TRN2", target_bir_lowering=False)
    pf = nc.dram_tensor("pf", [nmax, D], dt, kind="ExternalInput")
    offs = nc.dram_tensor("offs", [P, T], mybir.dt.int32, kind="ExternalInput")
    banks = [
        nc.dram_tensor(f"out{k}", [out_rows, c], dt, kind="ExternalOutput")
        for k in range(nbanks)
    ]
    with tile.TileContext(nc) as tc:
        with (
            tc.tile_pool(name="io", bufs=bufs) as io_pool,
            tc.tile_pool(name="misc", bufs=1) as misc,
        ):
            offs_sb = misc.tile([P, T], mybir.dt.int32)
            sums_sb = misc.tile([P, T * c], dt)
            # scalar engine HW queue: lands early, not behind the first
            # data tiles on the sync queue
            nc.scalar.dma_start(out=offs_sb[:], in_=offs[:])
            for base, s in sched:
                rows = slice(base * P, (base + s) * P)
                sb = io_pool.tile([P, s * D], dt, tag=f"sb{s}",
                                  bufs=bufs if s == sup else TBUFS)
                # pillar j = base*128 + p*s + blk -> partition p; host
                # pre-transposed each chunk so partition p's cw*D stretch
                # is [q (32 points), blk (cw pillars), c (64 ch)]
                nc.sync.dma_start(
                    out=sb[:],
                    in_=pf[rows, :].rearrange("(p x) w -> p (x w)", x=s),
                )
                for c0, cw in chunks_of(s, chunk):
                    o0 = c0 * D
                    # 5-level halving-add tree for this chunk, contiguous
                    # bf16 ops (DVE 2x perf mode, <=2048 elems each)
                    w = cw * D // 2
                    while w > cw * c:
                        nc.vector.tensor_add(
                            out=sb[:, o0:o0 + w],
                            in0=sb[:, o0:o0 + w],
                            in1=sb[:, o0 + w:o0 + 2 * w],
                        )
                        w //= 2
                    nc.vector.tensor_add(
                        out=sums_sb[:, (base + c0) * c:(base + c0 + cw) * c],
                        in0=sb[:, o0:o0 + w],
                        in1=sb[:, o0 + w:o0 + 2 * w],
                    )
                    # one indirect DMA per 128-pillar block (HW: one
                    # offset per partition), reading the decoupled sums
                    for blk in range(c0, c0 + cw):
                        g = base + blk
                        nc.gpsimd.indirect_dma_start(
                            out=banks[g % nbanks][:],
                            out_offset=bass.IndirectOffsetOnAxis(
                                ap=offs_sb[:, g:g + 1], axis=0
                            ),
                            in_=sums_sb[:, g * c:(g + 1) * c],
                            in_offset=None,
                        )
    nc.finalize()  # Bacc.compile(): splits multi-waits for TRN2 codegen
    return nc


def shard_inputs(point_features, voxel_coords, sup=SUP, taper=TAPER,
                 bf16=BF16, chunk=CHUNK):
    pf = np.ascontiguousarray(
        np.asarray(point_features, dtype=np.float32).reshape(N_PILLARS, N_POINTS * C)
    )
    np_dt = ml_dtypes.bfloat16 if bf16 else np.float32
    vc = np.asarray(voxel_coords)
    b = vc[:, 0].astype(np.int64)
    y = vc[:, 2].astype(np.int64)
    x = vc[:, 3].astype(np.int64)
    upper = (y >= HALF).astype(np.int64)
    region = b * 2 + upper
    off = (y - upper * HALF) * W + x  # row offset within the owned region
    idx_r = [np.nonzero(region == r)[0] for r in range(N_CORES)]
    nmax = max(len(ix) for ix in idx_r)
    nmax = max(P, ((nmax + P - 1) // P) * P)
    if not taper:
        a2 = sup * P
        nmax = ((nmax + a2 - 1) // a2) * a2
    T = nmax // P
    sched = make_schedule(T, sup, taper)
    inv_np = np.float32(1.0 / N_POINTS)
    in_maps = []
    for r in range(N_CORES):
        ix = idx_r[r]
        pf_r = np.zeros((nmax, N_POINTS * C), np_dt)
        # fold the mean's 1/n_points into the gather
        pf_r[: len(ix)] = (pf[ix] * inv_np).astype(np_dt)
        offs_r = np.full(nmax, REGION_ROWS, np.int32)  # pad rows -> dump row
        offs_r[: len(ix)] = off[ix].astype(np.int32)
        # tile (base, s): pillar j = base*128 + p*s + blk -> offs_arr[p, base+blk]
        offs_arr = np.empty((P, T), np.int32)
        for base, s in sched:
            seg = offs_r[base * P:(base + s) * P].reshape(P, s)
            offs_arr[:, base:base + s] = seg
            # per chunk-of-cw blocks: transpose partition p's pillars
            # from [blk, q, c] to [q, blk, c] for contiguous DVE trees
            blockrows = pf_r[base * P:(base + s) * P].reshape(P, s, N_POINTS * C)
            for c0, cw in chunks_of(s, chunk):
                grp = blockrows[:, c0:c0 + cw].reshape(P, cw, N_POINTS, C)
                blockrows[:, c0:c0 + cw] = (
                    grp.transpose(0, 2, 1, 3).reshape(P, cw, N_POINTS * C)
                )
        in_maps.append({"pf": pf_r, "offs": np.ascontiguousarray(offs_arr)})
    return in_maps, nmax


def assemble(results):
    out = np.empty((B, C, H, W), np.float32)
    for r in range(N_CORES):
        names = sorted(results[r])       # out0..out{nbanks-1}
        region = np.asarray(results[r][names[0]], dtype=np.float32)
        for name in names[1:]:
            # banks: disjoint row support
            region = region + np.asarray(results[r][name], dtype=np.float32)
        o = region[:REGION_ROWS].reshape(HALF, W, C)
        b_, half = divmod(r, 2)
        out[b_, :, half * HALF:(half + 1) * HALF, :] = o.transpose(2, 0, 1)
    return out


def run(point_features, voxel_coords, trace=False, sup=SUP, bufs=BUFS,
        nbanks=NBANKS, taper=TAPER, bf16=BF16, chunk=CHUNK, **spmd_kwargs):
    in_maps, nmax = shard_inputs(point_features, voxel_coords,
                                 sup=sup, taper=taper, bf16=bf16, chunk=chunk)
    nc = build_nc(nmax, sup=sup, bufs=bufs, nbanks=nbanks, taper=taper,
                  bf16=bf16, chunk=chunk)
    br = run_bass_kernel_spmd(
        nc, in_maps, list(range(N_CORES)), trace=trace, **spmd_kwargs
    )
    return assemble(br.results), br


def kernel(point_features, voxel_coords):
    out, _ = run(point_features, voxel_coords)
    return out
